# revision 32
# baseline (speedup 1.0000x reference)
"""Trainium2 Bass kernel for nn_DGProjectionBatchSparsity.

Computes: logits = x @ W.T (+b); per output neuron, mask of the top-k
(k=204) logits across the batch (4096). Output equals the mask numerically
(the per-neuron bias never changes within-neuron ranking, so it is ignored).

Sharding: column-parallel over out_features - each of the 8 cores owns a
1024-neuron slab, split into 8 o-tiles of 128 neurons (partition dim).

GEMM: float32r matmuls (1 cycle/row at free dim 512 vs 4 for f32).
float32r keeps 11 mantissa bits of each operand, so the host splits
W = W11 + Wlo (11-bit head + tail) and the kernel accumulates
x*W11 + x*Wlo in the same PSUM group: the W-side quantization error
cancels exactly and only the x-side f32r rounding (~3e-5 absolute on
logits with std 0.58) remains, keeping boundary-rank flips vs the fp32
reference well inside the correctness budget.

Per o-tile pipeline:
  1. PE GEMM into PSUM, 8 chunks of [128 x 512] over 4+4 k-tile matmuls.
  2. ACT epilogue: copy PSUM -> SBUF f32 logits (high priority, frees
     PSUM banks for the next chunks).
  3. c0 = #{x >= t0} via ACT Sign+accum in two halves; t0 = mu + z0*sigma
     from host-side seeds (exact per-neuron empirical mu/sigma via the
     Gram matrix of x - O(n d^2) host flops, tiny next to the O(n d m)
     device GEMM).
  4. Curvature-corrected Newton step -> t2 aiming at count K + BIAS
     (small per-neuron ops on DVE).
  5. Quarter-pipelined endgame: pen = (x < t2)*-2^100 on Pool, then
     zB = pen - x in place on DVE with accum (zB = -x above t2, else
     ~-2^100; the sum-of-x part of the accum rounds away next to the
     2^100 multiples, so acc = -2^100 * #below exactly), then top-8 of
     zB per 512-region (DVE max8).
  6. Merge regions to the top-W window via max8+match_replace rounds;
     idx = clip(c2-K, 0, W-1) selects -T via an iota-compare (T is the
     exact K-th largest logit: zB holds exact -x values).
  7. mask = x >= T_minus (T_minus one ulp below T so x == T is included),
     sliced across ACT (Sign->u8) and Pool (is_ge->f32, u8 unsupported
     there); host converts to the final f32 mask.
"""

import math

import numpy as np

import concourse.bass as bass
import concourse.tile as tile
from concourse import mybir
from concourse.bass_utils import run_bass_kernel_spmd

# ---------------------------------------------------------------- constants
BATCH = 4096
IN = 512
OUT = 8192
NCORES = 8
OSHARD = OUT // NCORES          # 1024 neurons per core
NTILES = OSHARD // 128          # 8 o-tiles per core
KTILES = IN // 128              # 4 contraction tiles
BCHUNK = 512
NBCH = BATCH // BCHUNK          # 8 batch chunks
K = max(1, int(0.05 * BATCH))   # 204

Z0 = 1.6467503276689657                      # Phi^-1(1 - K/BATCH)
PHI_Z0 = math.exp(-0.5 * Z0 * Z0) / math.sqrt(2.0 * math.pi)

BIAS = 12                       # aim count at K + BIAS so r = c2-K >= 0
WWIN = 32                       # selection window size (r in [0, WWIN-1])
NREG = 8                        # max8 regions per 4096 (512 cols each)
KTGT = float(K + BIAS)
BIGP = float(2.0 ** 100)
RBIGP = float(2.0 ** -100)
ONE_MEPS = float(1.0 - 2.0 ** -23)

F32 = mybir.dt.float32
F32R = mybir.dt.float32r
U8 = mybir.dt.uint8
ALU = mybir.AluOpType
ACTF = mybir.ActivationFunctionType

import os as _os
# mask slices: [0:MA] ACT->u8, [MA:MB] DVE->u8, [MB:] Pool->f32
MA = int(_os.environ.get("K_MA", 1536))
MB = int(_os.environ.get("K_MB", 3072))
LOGITS_BUFS = int(_os.environ.get("K_LOGITS_BUFS", 3))
WORK_BUFS = int(_os.environ.get("K_WORK_BUFS", 2))
SMALL_BUFS = 2
EPI_PRIO_OFFSET = int(_os.environ.get("K_EPI_PRIO", 40))
C0_PRIO = int(_os.environ.get("K_C0_PRIO", 0))
MASK_PRIO = int(_os.environ.get("K_MASK_PRIO", -4000))
DMA_BY_CHUNK = int(_os.environ.get("K_DMA_BY_CHUNK", 1))
# EXACT=1: 3-term split GEMM - x11*W11 + xlo*W + x11*Wlo, all float32r
# accumulating into the same PSUM. f32r keeps 11 mantissa bits of each
# operand exactly, so with x11/W11 pre-rounded to 11 bits host-side the
# main term is exact and the corrections carry the residual exactly
# (their own f32r rounding is O(2^-23) relative). PE cost 3x, but the
# mask then matches the fp32 reference to ~1e-6.
EXACT = int(_os.environ.get("K_EXACT", 1))

# -------------------------------------------- multi-wait split post-pass
# This container's walrus build lowers at most ONE semaphore wait per
# instruction (setupSyncWait asserts otherwise). Hoist extra waits onto
# same-engine NOPs inserted immediately before the instruction; per-engine
# program order makes this semantically identical.
from concourse.tile import TileContext
import bass_rust


def _split_multi_waits(nc):
    count = [0]

    def fresh():
        count[0] += 1
        return f"I-msw{count[0]}"

    for f in nc.m.functions:
        for bb in f.blocks:
            out = []
            changed = False
            for inst in bb.instructions:
                si = inst.sync_info
                if si is not None and si.on_wait and len(si.on_wait) > 1:
                    waits = list(si.on_wait)
                    for w in waits[:-1]:
                        nop = bass_rust.InstNoOp(name=fresh(), hint=None)
                        nop.engine = inst.engine
                        nop.sync_info = mybir.SyncInfo(on_wait=[w],
                                                       on_update=[])
                        out.append(nop)
                    si.on_wait = [waits[-1]]
                    changed = True
                out.append(inst)
            if changed:
                bb.instructions = out


# ---------------------------------------------------------------- program
def build_program():
    nc = bass.Bass("TRN2", target_bir_lowering=False, debug=False,
                   num_devices=NCORES)
    xT = nc.declare_dram_parameter("xT", [IN, BATCH], F32R, isOutput=False)
    wT = nc.declare_dram_parameter("wT", [IN, OSHARD], F32R, isOutput=False)
    if EXACT:
        wloT = nc.declare_dram_parameter("wloT", [IN, OSHARD], F32R,
                                         isOutput=False)
    else:
        wloT = None
    # seeds columns (NTILES each): t0 | negt0 | rls0 | hcurv
    seeds = nc.declare_dram_parameter("seeds", [128, 4 * NTILES], F32,
                                      isOutput=False)
    iota = nc.declare_dram_parameter("iota", [128, WWIN], F32,
                                     isOutput=False)
    mask_out = nc.declare_dram_parameter("mask", [OSHARD, MB], U8,
                                         isOutput=True)
    maskf_out = nc.declare_dram_parameter("maskf", [OSHARD, BATCH - MB], F32,
                                          isOutput=True)

    with TileContext(nc) as tc:
        _emit(nc, tc, xT, wT, wloT, seeds, iota, mask_out, maskf_out)
    _split_multi_waits(nc)
    return nc


def _emit(nc, tc, xT, wT, wloT, seeds, iota, mask_out, maskf_out):
    import contextlib
    ctx = contextlib.ExitStack()
    with ctx:
        resident = ctx.enter_context(tc.tile_pool(name="resident", bufs=1))
        logits_p = ctx.enter_context(tc.tile_pool(name="logits",
                                                  bufs=LOGITS_BUFS))
        work_p = ctx.enter_context(tc.tile_pool(name="work", bufs=WORK_BUFS))
        small_p = ctx.enter_context(tc.tile_pool(name="small",
                                                 bufs=SMALL_BUFS))
        psum_p = ctx.enter_context(
            tc.tile_pool(name="psum", bufs=8, space="PSUM"))

        # ---- resident inputs
        xTr = xT.rearrange("(ko p) b -> p ko b", p=128)
        wTr = wT.rearrange("(ko p) o -> p ko o", p=128)
        xt = []
        wt = []
        for kt in range(KTILES):
            wk = resident.tile([128, OSHARD], F32R, tag=f"wt{kt}",
                               name=f"wt{kt}")
            nc.sync.dma_start(wk[:], wTr[:, kt])
            wt.append(wk)
        for kt in range(KTILES):
            xk = resident.tile([128, BATCH], F32R, tag=f"xt{kt}",
                               name=f"xt{kt}")
            xt.append(xk)
        # chunk-granular loads so the first GEMM chunks start early;
        # DMA_BY_CHUNK orders all k-tiles of chunk 0 first so the first
        # matmul group can begin after 4 transfers instead of 28
        if DMA_BY_CHUNK:
            for bc in range(NBCH):
                for kt in range(KTILES):
                    nc.sync.dma_start(
                        xt[kt][:, bc * BCHUNK:(bc + 1) * BCHUNK],
                        xTr[:, kt, bc * BCHUNK:(bc + 1) * BCHUNK])
        else:
            for kt in range(KTILES):
                for bc in range(NBCH):
                    nc.sync.dma_start(
                        xt[kt][:, bc * BCHUNK:(bc + 1) * BCHUNK],
                        xTr[:, kt, bc * BCHUNK:(bc + 1) * BCHUNK])
        wlo = []
        if EXACT:
            wloTr = wloT.rearrange("(ko p) o -> p ko o", p=128)
            for kt in range(KTILES):
                wlk = resident.tile([128, OSHARD], F32R, tag=f"wlo{kt}",
                                    name=f"wlo{kt}")
                nc.sync.dma_start(wlk[:], wloTr[:, kt])
                wlo.append(wlk)
        seeds_t = resident.tile([128, 4 * NTILES], F32, tag="seeds")
        nc.sync.dma_start(seeds_t[:], seeds[:, :])
        iota_t = resident.tile([128, WWIN], F32, tag="iota")
        nc.sync.dma_start(iota_t[:], iota[:, :])

        for ot in range(NTILES):
            _emit_tile(nc, tc, xt, wt, wlo, seeds_t, iota_t, mask_out,
                       maskf_out, ot, logits_p, work_p, small_p, psum_p)


def _emit_tile(nc, tc, xt, wt, wlo, seeds_t, iota_t, mask_out,
               maskf_out, ot, logits_p, work_p, small_p, psum_p):
    v = nc.vector
    g = nc.gpsimd
    sc = nc.scalar

    t0 = seeds_t[:, ot:ot + 1]
    negt0 = seeds_t[:, NTILES + ot:NTILES + ot + 1]
    rls0 = seeds_t[:, 2 * NTILES + ot:2 * NTILES + ot + 1]
    hcurv = seeds_t[:, 3 * NTILES + ot:3 * NTILES + ot + 1]

    logits = logits_p.tile([128, BATCH], F32, tag="logits")
    pen = work_p.tile([128, BATCH], F32, tag="pen")
    masku = work_p.tile([128, MB], U8, tag="masku", bufs=2)
    maskf = work_p.tile([128, BATCH - MB], F32, tag="maskf", bufs=2)

    def tiny(tag):
        return small_p.tile([128, 1], F32, tag=tag, name=tag)

    o_lo = ot * 128

    # ---- GEMM (float32r) + ACT epilogue
    pss = [psum_p.tile([128, BCHUNK], F32, tag="ps", name=f"ps{bc}")
           for bc in range(NBCH)]
    for bc in range(NBCH):
        for kt in range(KTILES):
            nc.tensor.matmul(
                pss[bc][:],
                wt[kt][:, o_lo:o_lo + 128],
                xt[kt][:, bc * BCHUNK:(bc + 1) * BCHUNK],
                start=(kt == 0),
                stop=False if EXACT else (kt == KTILES - 1),
            )
        if EXACT:
            for kt in range(KTILES):
                nc.tensor.matmul(
                    pss[bc][:],
                    wlo[kt][:, o_lo:o_lo + 128],
                    xt[kt][:, bc * BCHUNK:(bc + 1) * BCHUNK],
                    start=False, stop=(kt == KTILES - 1),
                )
    for bc in range(NBCH):
        b_lo = bc * BCHUNK
        with tc.high_priority(offset=EPI_PRIO_OFFSET):
            sc.activation(logits[:, b_lo:b_lo + BCHUNK], pss[bc][:],
                          ACTF.Copy)

    # ---- c0 = #{x >= t0} on ACT in two halves (Sign output is junk)
    c0p = small_p.tile([128, 2], F32, tag="c0p")
    H = BATCH // 2
    with tc.high_priority(offset=C0_PRIO):
        sc.activation(pen[:, 0:H], logits[:, 0:H], ACTF.Sign, bias=negt0,
                      accum_out=c0p[:, 0:1])
        sc.activation(pen[:, H:], logits[:, H:], ACTF.Sign, bias=negt0,
                      accum_out=c0p[:, 1:2])
    # c0 = (ssumA + ssumB + BATCH)/2 ; Newton smalls on DVE
    c0 = tiny("c0")
    v.tensor_tensor(c0[:], c0p[:, 0:1], c0p[:, 1:2], ALU.add)
    v.tensor_scalar(c0[:], c0[:], float(BATCH), 0.5, ALU.add, ALU.mult)

    # ---- curvature-corrected Newton: t2 = t0 + d*(1 + hcurv*d)
    d = tiny("d")
    v.tensor_scalar(d[:], c0[:], -KTGT, rls0, ALU.add, ALU.mult)
    f = tiny("f")
    v.tensor_scalar(f[:], d[:], hcurv, 1.0, ALU.mult, ALU.add)
    step = tiny("step")
    v.tensor_tensor(step[:], d[:], f[:], ALU.mult)
    t2 = tiny("t2")
    v.tensor_tensor(t2[:], t0, step[:], ALU.add)

    # ---- quarter-pipelined: q=(x<t2)*-2^100 (Pool) -> zB=pen-x w/ accum
    # (DVE) -> max8 regions (DVE). The -x part of each accum is rounded
    # away next to the 2^100 multiples, so the accums recover the exact
    # count: acc = -2^100 * nbelow.
    RSZ = BATCH // NREG
    m64 = small_p.tile([128, NREG * 8], F32, tag="m64")
    accq = small_p.tile([128, 4], F32, tag="accq")
    QW = BATCH // 4
    for qt in range(4):
        qs = slice(QW * qt, QW * (qt + 1))
        g.tensor_scalar(pen[:, qs], logits[:, qs], t2[:], -BIGP,
                        ALU.is_lt, ALU.mult)
        v.scalar_tensor_tensor(pen[:, qs], pen[:, qs], 0.0, logits[:, qs],
                               ALU.add, ALU.subtract,
                               accum_out=accq[:, qt:qt + 1])
        for j in (2 * qt, 2 * qt + 1):
            v.max(m64[:, 8 * j:8 * j + 8], pen[:, RSZ * j:RSZ * (j + 1)])

    # c2 = BATCH - nbelow; idx = clip(c2-K, 0, W-1) = clip(acc*2^-100 +
    # BATCH - K, ...)
    acc = tiny("acc")
    v.tensor_tensor(acc[:], accq[:, 0:1], accq[:, 1:2], ALU.add)
    acc2 = tiny("acc2")
    v.tensor_tensor(acc2[:], accq[:, 2:3], accq[:, 3:4], ALU.add)
    v.tensor_tensor(acc[:], acc[:], acc2[:], ALU.add)
    idx = tiny("idx")
    v.tensor_scalar(idx[:], acc[:], RBIGP, float(BATCH - K), ALU.mult,
                    ALU.add)
    v.tensor_scalar(idx[:], idx[:], 0.0, float(WWIN - 1), ALU.max, ALU.min)
    mw = small_p.tile([128, WWIN], F32, tag="mw")
    nrounds = WWIN // 8
    for i in range(nrounds):
        v.max(mw[:, 8 * i:8 * i + 8], m64[:])
        if i < nrounds - 1:
            v.match_replace(m64[:], in_to_replace=mw[:, 8 * i:8 * i + 8],
                            in_values=m64[:], imm_value=-BIGP)

    # ---- select the idx-th (0-based) largest of mw -> ysel = -T
    selm = small_p.tile([128, WWIN], F32, tag="selm")
    v.tensor_scalar(selm[:], iota_t[:], idx[:], None, ALU.is_equal)
    v.tensor_tensor(selm[:], selm[:], mw[:], ALU.mult)
    ysel = tiny("ysel")
    v.reduce_sum(ysel[:], selm[:], axis=mybir.AxisListType.X)
    # mask bias = -T_minus = ysel*(1-2^-23); T_minus = -bias for is_ge
    negTm = tiny("negTm")
    v.tensor_scalar(negTm[:], ysel[:], ONE_MEPS, None, ALU.mult)
    Tm = tiny("Tm")
    v.tensor_scalar(Tm[:], ysel[:], -ONE_MEPS, None, ALU.mult)

    # ---- mask slices: ACT u8 halves, Pool f32; DMA out per slice.
    # Low priority: these consume the late select result, and must not
    # block the next tiles' front-end work on their engines.
    with tc.high_priority(offset=MASK_PRIO):
        sc.activation(masku[:, 0:MA], logits[:, 0:MA], ACTF.Sign,
                      bias=negTm[:])
        nc.sync.dma_start(mask_out[o_lo:o_lo + 128, 0:MA], masku[:, 0:MA])
        if MB > MA:
            v.tensor_scalar(masku[:, MA:MB], logits[:, MA:MB], Tm[:], 0.0,
                            ALU.is_ge, ALU.add)
            nc.sync.dma_start(mask_out[o_lo:o_lo + 128, MA:MB],
                              masku[:, MA:MB])
        g.tensor_scalar(maskf[:], logits[:, MB:], Tm[:], 0.0, ALU.is_ge,
                        ALU.add)
        nc.sync.dma_start(maskf_out[o_lo:o_lo + 128, :], maskf[:])


# ---------------------------------------------------------------- host API
_CACHE = {}


def _host_seeds(x, W):
    """Per-neuron t0/rls0/hcurv from exact empirical mu/sigma."""
    xd = x.astype(np.float64)
    Wd = W.astype(np.float64)
    sx = xd.sum(0)
    G = xd.T @ xd
    mu = (Wd @ sx) / BATCH
    ex2 = ((Wd @ G) * Wd).sum(1) / BATCH
    sig = np.sqrt(np.maximum(ex2 - mu * mu, 1e-12))
    t0 = (mu + Z0 * sig).astype(np.float32)
    rls0 = (sig / (BATCH * PHI_Z0)).astype(np.float32)
    hcurv = (0.5 * Z0 / sig).astype(np.float32)
    return t0, rls0, hcurv


def kernel(x=None, W=None, b=None, **_unused):
    x = np.ascontiguousarray(np.asarray(x, dtype=np.float32))
    W = np.ascontiguousarray(np.asarray(W, dtype=np.float32))
    assert x.shape == (BATCH, IN) and W.shape == (OUT, IN)

    nc = _CACHE.get("nc")
    if nc is None:
        nc = build_program()
        _CACHE["nc"] = nc

    t0, rls0, hcurv = _host_seeds(x, W)
    iota = np.ascontiguousarray(
        np.tile(np.arange(WWIN, dtype=np.float32), (128, 1)))

    xT = np.ascontiguousarray(x.T)
    if EXACT:
        u = W.view(np.uint32)
        W11 = ((u + np.uint32(1 << 11)) & np.uint32(0xFFFFF000)
               ).view(np.float32)
        Wlo = (W - W11).astype(np.float32)
    in_maps = []
    for c in range(NCORES):
        sl = slice(c * OSHARD, (c + 1) * OSHARD)
        seeds = np.empty((128, 4 * NTILES), np.float32)
        seeds[:, 0:NTILES] = t0[sl].reshape(NTILES, 128).T
        seeds[:, NTILES:2 * NTILES] = -t0[sl].reshape(NTILES, 128).T
        seeds[:, 2 * NTILES:3 * NTILES] = rls0[sl].reshape(NTILES, 128).T
        seeds[:, 3 * NTILES:4 * NTILES] = hcurv[sl].reshape(NTILES, 128).T
        im = {
            "xT": xT,
            "wT": np.ascontiguousarray((W11 if EXACT else W)[sl].T),
            "seeds": np.ascontiguousarray(seeds),
            "iota": iota,
        }
        if EXACT:
            im["wloT"] = np.ascontiguousarray(Wlo[sl].T)
        in_maps.append(im)
    res = run_bass_kernel_spmd(nc, in_maps, list(range(NCORES)))
    out = np.empty((BATCH, OUT), np.float32)
    for c in range(NCORES):
        m = res.results[c]["mask"]           # [OSHARD, MB] u8
        mf = res.results[c]["maskf"]         # [OSHARD, BATCH-MB] f32
        sl = slice(c * OSHARD, (c + 1) * OSHARD)
        out[0:MB, sl] = (m == 1).T.astype(np.float32)
        out[MB:, sl] = (mf >= 0.5).T.astype(np.float32)
    return out
